# revision 8
# baseline (speedup 1.0000x reference)
import os
import sys

for _p in ("/opt/trn_rl_repo", "/root/.axon_site/_ro/trn_rl_repo"):
    if os.path.isdir(_p) and _p not in sys.path:
        sys.path.insert(0, _p)

import numpy as np

import concourse.bass as bass
import concourse.mybir as mybir
from concourse.tile import TileContext
from concourse import bass_utils
from concourse import bacc

F32 = mybir.dt.float32
I32 = mybir.dt.int32
AF = mybir.ActivationFunctionType
OP = mybir.AluOpType

N_CORES = 8
BATCH = 65536
C = 4              # classes
T = 120            # time steps
PB = BATCH // N_CORES      # batch per core = 8192
G = 32             # partition groups per class (4*32 = 128 partitions)
FB = PB // G       # free-dim batch per partition = 256
CH = 4             # timesteps per DMA chunk
DT_MS = 10.0
EPS = 1e-9
# accumulator is stored scaled: S = 5*acc, so acc_new = max(acc+0.2*(sp-acc),0)
# becomes S_new = max(0.8*S + sp, 0) and the 0.5 threshold becomes 2.5.


def _softplus(x):
    return np.logaddexp(0.0, x.astype(np.float64)).astype(np.float32)


def _build(nc, w00, pb0, inh, ns, input_scale):
    noise_d = nc.dram_tensor("noise", [T // CH, 128, CH * FB], F32, kind="ExternalInput")
    logits_d = nc.dram_tensor("logits_t", [128, FB], F32, kind="ExternalInput")
    w_d = nc.dram_tensor("wmat", [128, 128], F32, kind="ExternalInput")
    out_d = nc.dram_tensor("out", [128, FB], F32, kind="ExternalOutput")

    with TileContext(nc) as tc:
        with (
            tc.tile_pool(name="persist", bufs=1) as persist,
            tc.tile_pool(name="noise", bufs=3) as npool,
            tc.tile_pool(name="work", bufs=2) as work,
            tc.tile_pool(name="psum", bufs=2, space="PSUM") as psum,
        ):
            Wt0 = persist.tile([128, 128], F32)
            nc.sync.dma_start(Wt0[:], w_d[:])
            Wt = persist.tile([128, 128], F32)
            nc.vector.tensor_copy(Wt[:], Wt0[:])
            ev = persist.tile([128, FB], F32)
            lg = persist.tile([128, FB], F32)
            nc.sync.dma_start(lg[:], logits_d[:])
            # evidence = relu(logits*input_scale)*w00 + pb0
            nc.scalar.activation(ev[:], lg[:], AF.Relu, scale=float(input_scale))
            nc.vector.tensor_scalar(ev[:], ev[:], float(w00), float(pb0), OP.mult, OP.add)

            Sa = persist.tile([128, FB], F32)
            Sb = persist.tile([128, FB], F32)
            found = persist.tile([128, FB], I32)
            cnt = persist.tile([128, FB], F32)
            Sp = persist.tile([128, FB], F32)
            Sn = persist.tile([128, FB], F32)
            for tl in (Sa, Sb, found, cnt, Sp, Sn):
                nc.vector.memset(tl[:], 0.0)

            cur, nxt = Sa, Sb
            for ci in range(T // CH):
                ntile = npool.tile([128, CH * FB], F32)
                nc.sync.dma_start(ntile[:], noise_d[ci])
                for ti in range(CH):
                    t = ci * CH + ti
                    nslice = ntile[:, ti * FB:(ti + 1) * FB]
                    # z = alpha/5 * S - inh/5 * sum_classes(S), via one PE matmul
                    z = psum.tile([128, FB], F32)
                    nc.tensor.matmul(z[:], Wt[:], cur[:], start=True, stop=True)
                    t1 = work.tile([128, FB], F32, tag="t1")
                    nc.vector.scalar_tensor_tensor(t1[:], nslice, float(ns), z[:], OP.mult, OP.add)
                    drive = work.tile([128, FB], F32, tag="drive")
                    nc.vector.tensor_add(drive[:], t1[:], ev[:])
                    ex = work.tile([128, FB], F32, tag="ex")
                    nc.scalar.activation(ex[:], drive[:], AF.Exp)
                    sp = work.tile([128, FB], F32, tag="sp")
                    nc.scalar.activation(sp[:], ex[:], AF.Ln, bias=1.0)
                    u = work.tile([128, FB], F32, tag="u")
                    nc.vector.scalar_tensor_tensor(u[:], cur[:], 0.8, sp[:], OP.mult, OP.add)
                    newly = work.tile([128, FB], I32, tag="newly")
                    nc.vector.scalar_tensor_tensor(newly[:], u[:], 2.5, found[:], OP.is_ge, OP.is_gt)
                    nc.vector.copy_predicated(Sp[:], newly[:], cur[:])
                    nc.vector.copy_predicated(Sn[:], newly[:], u[:])
                    nc.vector.tensor_add(found[:], found[:], newly[:])
                    nc.vector.tensor_add(cnt[:], cnt[:], found[:])
                    nc.scalar.activation(nxt[:], u[:], AF.Relu)
                    cur, nxt = nxt, cur

            # idx = T - cnt; idx0 = max(idx-1, 0)
            idx = work.tile([128, FB], F32, tag="t1")
            nc.vector.tensor_scalar(idx[:], cnt[:], -1.0, float(T), OP.mult, OP.add)
            idx0 = work.tile([128, FB], F32, tag="drive")
            nc.vector.tensor_scalar(idx0[:], idx[:], 1.0, 0.0, OP.subtract, OP.max)
            # frac = (2.5 - Sp) / (Sn - Sp + 5*EPS), zeroed when idx == 0
            den = work.tile([128, FB], F32, tag="sp")
            nc.vector.tensor_sub(den[:], Sn[:], Sp[:])
            nc.vector.tensor_scalar(den[:], den[:], 5.0 * EPS, None, OP.add)
            rec = work.tile([128, FB], F32, tag="u")
            nc.vector.reciprocal(rec[:], den[:])
            num = work.tile([128, FB], F32, tag="num")
            nc.vector.tensor_scalar(num[:], Sp[:], -1.0, 2.5, OP.mult, OP.add)
            frac = work.tile([128, FB], F32, tag="frac")
            nc.vector.tensor_mul(frac[:], num[:], rec[:])
            mi = work.tile([128, FB], F32, tag="mi")
            nc.vector.tensor_scalar(mi[:], idx[:], 0.5, None, OP.is_ge)
            nc.vector.tensor_mul(frac[:], frac[:], mi[:])
            tval = work.tile([128, FB], F32, tag="tval")
            nc.vector.tensor_add(tval[:], idx0[:], frac[:])
            # out_sec = found ? tval*DT/1000 : T*DT/1000
            tmax = T * DT_MS / 1000.0
            nc.vector.tensor_scalar(tval[:], tval[:], DT_MS / 1000.0, -tmax, OP.mult, OP.add)
            nc.vector.tensor_mul(tval[:], tval[:], found[:])
            nc.vector.tensor_scalar(tval[:], tval[:], tmax, None, OP.add)
            nc.sync.dma_start(out_d[:], tval[:])
    return nc


last_results = None


def kernel(logits, input_scale, leak, self_excitation, inhibition, noise_std,
           proj_w, proj_b, noise_base):
    logits = np.asarray(logits, dtype=np.float32)
    noise_base = np.asarray(noise_base, dtype=np.float32)
    lk = _softplus(np.asarray(leak))
    se = _softplus(np.asarray(self_excitation))
    inh = float(_softplus(np.asarray(inhibition)))
    ns = float(_softplus(np.asarray(noise_std)))
    alpha = se + inh - lk  # [C]
    w00 = float(np.asarray(proj_w)[0, 0])
    pb0 = float(np.asarray(proj_b)[0])
    iscale = float(np.asarray(input_scale))

    # W[p,q] = (alpha[class(q)]*(p==q) - inh*(p%G==q%G)) / 5
    p_idx = np.arange(128)
    q_idx = np.arange(128)
    Wm = (-inh / 5.0) * (p_idx[:, None] % G == q_idx[None, :] % G).astype(np.float32)
    Wm[q_idx, q_idx] += alpha[q_idx // G] / 5.0

    nc = bacc.Bacc("TRN2", target_bir_lowering=False, debug=False, num_devices=N_CORES)
    _build(nc, w00, pb0, inh, ns, iscale)
    nc.compile()

    in_maps = []
    for c in range(N_CORES):
        s = c * PB
        nz = noise_base[:, s:s + PB, :].reshape(T, G, FB, C)
        nz = np.ascontiguousarray(nz.transpose(0, 3, 1, 2)).reshape(T, 128, FB)
        nz = np.ascontiguousarray(
            nz.reshape(T // CH, CH, 128, FB).transpose(0, 2, 1, 3)
        ).reshape(T // CH, 128, CH * FB)
        lg = logits[s:s + PB].reshape(G, FB, C)
        lg = np.ascontiguousarray(lg.transpose(2, 0, 1)).reshape(128, FB)
        in_maps.append({"noise": nz, "logits_t": lg, "wmat": Wm})

    res = bass_utils.run_bass_kernel_spmd(nc, in_maps, core_ids=list(range(N_CORES)))
    global last_results
    last_results = res
    outs = []
    for c in range(N_CORES):
        o = res.results[c]["out"].reshape(C, G, FB)
        outs.append(o.transpose(1, 2, 0).reshape(PB, C))
    return np.concatenate(outs, axis=0)


# revision 24
# speedup vs baseline: 279.1999x; 279.1999x over previous
import os
import sys

for _p in ("/opt/trn_rl_repo", "/root/.axon_site/_ro/trn_rl_repo"):
    if os.path.isdir(_p) and _p not in sys.path:
        sys.path.insert(0, _p)

import numpy as np

import concourse.bass as bass
import concourse.mybir as mybir
from concourse.tile import TileContext
from concourse import bass_utils
from concourse import bacc

F32 = mybir.dt.float32
F32R = mybir.dt.float32r
I32 = mybir.dt.int32
AF = mybir.ActivationFunctionType
OP = mybir.AluOpType

N_CORES = 8
BATCH = 65536
C = 4              # classes
T = 120            # time steps
PB = BATCH // N_CORES      # batch per core = 8192
G = 32             # partition groups per class (4*32 = 128 partitions)
FB = PB // G       # free-dim batch per partition = 256
CH = 4             # timesteps per DMA chunk
NS = 1             # independent streams (free-dim split) to hide latency
SW = FB // NS      # stream width
DT_MS = 10.0
EPS = 1e-9
# accumulator is stored scaled: S = 5*acc, so acc_new = max(acc+0.2*(sp-acc),0)
# becomes S_new = max(0.8*S + sp, 0) and the 0.5 threshold becomes 2.5.


def _softplus(x):
    return np.logaddexp(0.0, x.astype(np.float64)).astype(np.float32)


def _build(nc, w00, pb0, inh, ns, input_scale):
    noise_d = nc.dram_tensor("noise", [T // CH, 128, CH * FB], F32, kind="ExternalInput")
    logits_d = nc.dram_tensor("logits_t", [128, FB], F32, kind="ExternalInput")
    w_d = nc.dram_tensor("wmat", [128, 128], F32, kind="ExternalInput")
    out_d = nc.dram_tensor("out", [128, FB], F32, kind="ExternalOutput")

    with TileContext(nc) as tc:
        with (
            tc.tile_pool(name="persist", bufs=1) as persist,
            tc.tile_pool(name="noise", bufs=3) as npool,
            tc.tile_pool(name="work", bufs=3) as work,
            tc.tile_pool(name="psum", bufs=6, space="PSUM") as psum,
        ):
            Wt0 = persist.tile([128, 128], F32)
            nc.sync.dma_start(Wt0[:], w_d[:])
            Wt = persist.tile([128, 128], F32)
            nc.vector.tensor_copy(Wt[:], Wt0[:])
            ev = persist.tile([128, FB], F32)
            lg = persist.tile([128, FB], F32)
            nc.sync.dma_start(lg[:], logits_d[:])
            # evidence = relu(logits*input_scale)*w00 + pb0
            nc.scalar.activation(ev[:], lg[:], AF.Relu, scale=float(input_scale))
            nc.vector.tensor_scalar(ev[:], ev[:], float(w00), float(pb0), OP.mult, OP.add)

            Scur = [persist.tile([128, SW], F32, name=f"Scur{i}") for i in range(NS)]
            Snxt = [persist.tile([128, SW], F32, name=f"Snxt{i}") for i in range(NS)]
            nf = [persist.tile([128, SW], I32, name=f"nf{i}") for i in range(NS)]
            cnt = [persist.tile([128, SW], I32, name=f"cnt{i}") for i in range(NS)]
            Sp = [persist.tile([128, SW], F32, name=f"Sp{i}") for i in range(NS)]
            Sn = [persist.tile([128, SW], F32, name=f"Sn{i}") for i in range(NS)]
            for tls in (Scur, Snxt, cnt, Sp, Sn):
                for tl in tls:
                    nc.vector.memset(tl[:], 0.0)
            for tl in nf:
                nc.vector.memset(tl[:], 1)

            def bookkeeping(s, scur, u, par=0):
                # First-crossing capture: while nf (not-found) is 1, Sp/Sn
                # shadow the pre/post state; nf drops to 0 at the first
                # crossing, freezing them. cnt = sum of nf = crossing index.
                nc.vector.copy_predicated(Sp[s][:], nf[s][:], scur[:])
                nc.vector.copy_predicated(Sn[s][:], nf[s][:], u[:])
                nc.vector.scalar_tensor_tensor(nf[s][:], u[:], 2.5, nf[s][:], OP.is_lt, OP.mult)
                nc.gpsimd.tensor_add(cnt[s][:], cnt[s][:], nf[s][:])

            pend = [None] * NS
            spv = [None] * NS
            Yt = [persist.tile([128, SW], F32, name=f"Ya{i}") for i in range(NS)]
            Yn = [persist.tile([128, SW], F32, name=f"Yb{i}") for i in range(NS)]
            for s in range(NS):
                nc.vector.memset(Yt[s][:], 0.0)   # Ytilde_0 = 0
            for ci in range(T // CH):
                ntile = npool.tile([128, CH * FB], F32)
                nc.sync.dma_start(ntile[:], noise_d[ci])
                for ti in range(CH):
                    for s in range(NS):
                        t = ci * CH + ti
                        cur, nxt = Scur[s], Snxt[s]
                        nslice = ntile[:, ti * FB + s * SW: ti * FB + (s + 1) * SW]
                        evs = ev[:, s * SW:(s + 1) * SW]
                        # off-cycle precombine on Pool: pn = ns*noise + ev,
                        # pn2 = 0.8*Ytilde + pn
                        # noise comes ns-prescaled from the host reshard pass
                        pn = work.tile([128, SW], F32, tag=f"pn{s}", name=f"pn{s}")
                        nc.gpsimd.tensor_add(pn[:], nslice, evs)
                        drive = work.tile([128, SW], F32, tag=f"dr{s}", name=f"dr{s}")
                        if t > 0:
                            # z = W^T sp_{t-1} feeds both drive and Ytilde
                            z = psum.tile([128, SW], F32, tag=f"z{s}", name=f"z{s}")
                            nc.tensor.matmul(z[:], Wt[:], spv[s][:], start=True, stop=True)
                            pn2 = work.tile([128, SW], F32, tag=f"p2{s}", name=f"p2{s}")
                            nc.gpsimd.tensor_add(pn2[:], Yt[s][:], pn[:])
                            # W carries a 0.8 factor, so z' = 0.8*z: undo with 1.25
                            nc.vector.scalar_tensor_tensor(drive[:], z[:], 1.25, pn2[:], OP.mult, OP.add)
                        else:
                            nc.vector.tensor_copy(drive[:], pn[:])
                        if pend[s] is not None:
                            bookkeeping(s, *pend[s])
                        ex = work.tile([128, SW], F32, tag=f"ex{s}", name=f"ex{s}")
                        nc.scalar.activation(ex[:], drive[:], AF.Exp)
                        sp = work.tile([128, SW], F32, tag=f"sp{s}", name=f"sp{s}", bufs=3)
                        nc.scalar.activation(sp[:], ex[:], AF.Ln, bias=1.0)
                        spv[s] = sp
                        if t > 0:
                            # Ytilde_t = 0.8*Ytilde_{t-1} + z (off-cycle; feeds pn2_{t+1})
                            nc.vector.scalar_tensor_tensor(Yn[s][:], Yt[s][:], 0.8, z[:], OP.mult, OP.add)
                            Yt[s], Yn[s] = Yn[s], Yt[s]
                        # u = 0.8*S + sp IS the new state (never negative, the
                        # reference's max(.,0) is dead code) - off the cycle.
                        nc.vector.scalar_tensor_tensor(nxt[:], cur[:], 0.8, sp[:], OP.mult, OP.add)
                        pend[s] = (cur, nxt, t % 2)
                        Scur[s], Snxt[s] = nxt, cur
            for s in range(NS):
                bookkeeping(s, *pend[s])
            for s in range(NS):
                # idx = cnt (sum of not-found flags); idx0 = max(idx-1, 0)
                fnd = work.tile([128, SW], F32, tag=f"fd{s}")
                nc.vector.tensor_scalar(fnd[:], nf[s][:], -1.0, 1.0, OP.mult, OP.add)
                idx = work.tile([128, SW], F32, tag=f"t1{s}")
                nc.vector.tensor_scalar(idx[:], cnt[s][:], 1.0, None, OP.mult)
                idx0 = work.tile([128, SW], F32, tag=f"dr{s}")
                nc.vector.tensor_scalar(idx0[:], idx[:], 1.0, 0.0, OP.subtract, OP.max)
                # frac = (2.5 - Sp) / (Sn - Sp + 5*EPS), zeroed when idx == 0
                den = work.tile([128, SW], F32, tag=f"sp{s}")
                nc.vector.tensor_sub(den[:], Sn[s][:], Sp[s][:])
                nc.vector.tensor_scalar(den[:], den[:], 5.0 * EPS, None, OP.add)
                rec = work.tile([128, SW], F32, tag=f"u{s}")
                nc.vector.reciprocal(rec[:], den[:])
                num = work.tile([128, SW], F32, tag=f"nm{s}")
                nc.vector.tensor_scalar(num[:], Sp[s][:], -1.0, 2.5, OP.mult, OP.add)
                frac = work.tile([128, SW], F32, tag=f"fr{s}")
                nc.vector.tensor_mul(frac[:], num[:], rec[:])
                mi = work.tile([128, SW], F32, tag=f"mi{s}")
                nc.vector.tensor_scalar(mi[:], idx[:], 0.5, None, OP.is_ge)
                nc.vector.tensor_mul(frac[:], frac[:], mi[:])
                tval = work.tile([128, SW], F32, tag=f"tv{s}")
                nc.vector.tensor_add(tval[:], idx0[:], frac[:])
                # out_sec = found ? tval*DT/1000 : T*DT/1000
                tmax = T * DT_MS / 1000.0
                nc.vector.tensor_scalar(tval[:], tval[:], DT_MS / 1000.0, -tmax, OP.mult, OP.add)
                nc.vector.tensor_mul(tval[:], tval[:], fnd[:])
                nc.vector.tensor_scalar(tval[:], tval[:], tmax, None, OP.add)
                nc.sync.dma_start(out_d[:, s * SW:(s + 1) * SW], tval[:])
    return nc


def _pin_act_table(nc):
    # All activation funcs used (Exp, Ln, Relu, Copy) live together in the
    # natural_log_exp_and_others set; blank the others (keeping list indices,
    # which are the runtime set ids) so the chooser can't ping-pong tables
    # inside the scan loop.
    from concourse import hw_specs as _hs
    import concourse.bacc as _bacc
    full = dict(_hs.get_activation_tables(nc.m.arch))
    keep = "natural_log_exp_and_others"
    patched = {k: (v if k == keep else set()) for k, v in full.items()}
    _bacc.get_activation_tables = lambda arch: patched


last_results = None


def kernel(logits, input_scale, leak, self_excitation, inhibition, noise_std,
           proj_w, proj_b, noise_base):
    logits = np.asarray(logits, dtype=np.float32)
    noise_base = np.asarray(noise_base, dtype=np.float32)
    lk = _softplus(np.asarray(leak))
    se = _softplus(np.asarray(self_excitation))
    inh = float(_softplus(np.asarray(inhibition)))
    ns = float(_softplus(np.asarray(noise_std)))
    alpha = se + inh - lk  # [C]
    w00 = float(np.asarray(proj_w)[0, 0])
    pb0 = float(np.asarray(proj_b)[0])
    iscale = float(np.asarray(input_scale))

    # W[p,q] = (alpha[class(q)]*(p==q) - inh*(p%G==q%G)) / 5
    p_idx = np.arange(128)
    q_idx = np.arange(128)
    Wm = (-inh / 5.0) * (p_idx[:, None] % G == q_idx[None, :] % G).astype(np.float32)
    Wm[q_idx, q_idx] += alpha[q_idx // G] / 5.0
    Wm *= 0.8  # Ys-recurrence scaling: z' = 0.8*z

    nc = bacc.Bacc("TRN2", target_bir_lowering=False, debug=False, num_devices=N_CORES)
    _build(nc, w00, pb0, inh, ns, iscale)
    _pin_act_table(nc)
    nc.compile()

    in_maps = []
    for c in range(N_CORES):
        s = c * PB
        nz = noise_base[:, s:s + PB, :].reshape(T, G, FB, C) * np.float32(ns)
        nz = np.ascontiguousarray(nz.transpose(0, 3, 1, 2)).reshape(T, 128, FB)
        nz = np.ascontiguousarray(
            nz.reshape(T // CH, CH, 128, FB).transpose(0, 2, 1, 3)
        ).reshape(T // CH, 128, CH * FB)
        lg = logits[s:s + PB].reshape(G, FB, C)
        lg = np.ascontiguousarray(lg.transpose(2, 0, 1)).reshape(128, FB)
        in_maps.append({"noise": nz, "logits_t": lg, "wmat": Wm})

    res = bass_utils.run_bass_kernel_spmd(nc, in_maps, core_ids=list(range(N_CORES)))
    global last_results
    last_results = res
    outs = []
    for c in range(N_CORES):
        o = res.results[c]["out"].reshape(C, G, FB)
        outs.append(o.transpose(1, 2, 0).reshape(PB, C))
    return np.concatenate(outs, axis=0)
